# revision 7
# baseline (speedup 1.0000x reference)
"""GQA attention kernel for Trainium2, tensor-parallel over (batch, kv-head-pair).

Problem: B=2, S=2048, D=2048, 32 q heads / 8 kv heads, head_dim 64,
scores get an additive mask [1,1,S,S] + per-batch graph bias [B,1,S,S].

Sharding: 16 units = (batch 2) x (kv-head-pair 4) over 8 cores; core c handles
batch b = c % 2 and kv heads {2*(c//2), 2*(c//2)+1} (8 q heads). Each core
computes its heads' attention output and its slice of the wo matmul; the host
sums the 4 partial outputs per batch.

Per-core dataflow (everything "transposed" so no on-chip probs transpose is
ever needed):
  xT tiles   : PE-transpose of x            [din, s]
  xqT/xkT    : w.T @ x.T via PE             [(kvl,d), (rep, q)] / [(kvl,d), kpos]
  scoresT    : xkT.T-tile @ xqT  (K=64)     [kpos, (rep, q)]  (kv0/kv1 row-packed)
  probsT     : exp(s/8)*exp(mask+bias)      bf16, exp on ACT, mul on DVE
  attnT      : [xv|1].T @ probsT (K=128)    [d+1, (rep, q)], rowsum rides row 64
  y          : attnT-tile.T @ wo            [s, dmodel]
The mask+bias combine happens on host (pure input prep); its exp happens on
device. exp(s+c) = exp(s)*exp(c) keeps the DVE work in 2x-rate bf16 and the
additive bias out of the matmul path entirely.
"""

import sys

if "/opt/trn_rl_repo" not in sys.path:
    sys.path.insert(0, "/opt/trn_rl_repo")

import numpy as np
import ml_dtypes
from contextlib import ExitStack

import concourse.bass as bass
import concourse.tile as tile
from concourse import bacc, mybir
from concourse.bass_utils import run_bass_kernel_spmd
from concourse.masks import make_identity

F32 = mybir.dt.float32
BF16 = mybir.dt.bfloat16

D = 2048          # model dim
HD = 64           # head dim
NREP = 4          # q heads per kv head
NKVL = 2          # kv heads per core
N_CORES = 8
DOUT_Q = NREP * NKVL * HD   # 512
DOUT_KV = NKVL * HD         # 128
WCOLS = DOUT_Q + 2 * DOUT_KV  # 768


def build_program(S=2048, causal=False, loop_n=1):
    """Build the per-core Bass program. All 8 cores run the same program on
    different input shards. loop_n>1 wraps the body in a hardware loop for
    timing (the body is idempotent)."""
    G = S // 128   # q groups
    TK = S // 128  # kpos tiles
    NSC = S // 512 # s-chunks for the projection phase
    assert S % 512 == 0

    nc = bacc.Bacc("TRN2", target_bir_lowering=False, debug=False,
                   num_devices=N_CORES)
    x_d = nc.dram_tensor("x", (S, D), F32, kind="ExternalInput").ap()
    comb_d = nc.dram_tensor("comb", (S, S), BF16, kind="ExternalInput").ap()
    wqkv_d = nc.dram_tensor("wqkv", (D, WCOLS), F32, kind="ExternalInput").ap()
    wo_d = nc.dram_tensor("wo", (DOUT_Q, D), F32, kind="ExternalInput").ap()
    y_d = nc.dram_tensor("y", (S, D), F32, kind="ExternalOutput").ap()

    with tile.TileContext(nc) as tc, ExitStack() as ctx:
        const_pool = ctx.enter_context(tc.tile_pool(name="const", bufs=1))
        ident = const_pool.tile([128, 128], F32)
        make_identity(nc, ident)

        def body(_it=None):
            with ExitStack() as bctx:
                persist = bctx.enter_context(tc.tile_pool(name="persist", bufs=1))
                xqT = persist.tile([128, NREP * S], F32)      # [(kvl,d), (rep,q)]
                xkT = persist.tile([128, S], F32)             # [(kvl,d), kpos]
                xv0 = persist.tile([128, TK * 65], BF16)      # [kpos, (tk, d+1)]
                xv1 = persist.tile([128, TK * 65], BF16)
                attnT = persist.tile([128, NREP * S], F32)    # [(kvl,d), (rep,q)]
                xqT3 = xqT.rearrange("p (h q) -> p h q", h=NREP)
                attnT3 = attnT.rearrange("p (h q) -> p h q", h=NREP)
                xv0r = xv0.rearrange("p (t c) -> p t c", c=65)
                xv1r = xv1.rearrange("p (t c) -> p t c", c=65)
                nc.vector.memset(xv0r[:, :, 64:65], 1.0)
                nc.vector.memset(xv1r[:, :, 64:65], 1.0)

                # ---------------- Phase A: projections ----------------
                with tc.tile_pool(name="wq_pool", bufs=1) as wpool, \
                     tc.tile_pool(name="panel_pool", bufs=1) as panel_pool, \
                     tc.tile_pool(name="xnat_pool", bufs=2) as xnat_pool, \
                     tc.tile_pool(name="vfix_pool", bufs=2) as vfix_pool, \
                     tc.tile_pool(name="psA", bufs=2, space="PSUM") as psA, \
                     tc.tile_pool(name="psB", bufs=1, space="PSUM") as psB:
                    w_sb = wpool.tile([128, 16 * WCOLS], F32)
                    w3 = w_sb.rearrange("p (t o) -> p t o", t=16)
                    nc.sync.dma_start(w3, wqkv_d.rearrange("(t p) o -> p t o", p=128))

                    for sc in range(NSC):
                        panel = panel_pool.tile([128, 16 * 512], F32, tag="panel")
                        pv3 = panel.rearrange("p (t s) -> p t s", t=16)
                        for sb in range(4):
                            xn = xnat_pool.tile([128, D], F32, tag="xn")
                            r0 = sc * 512 + sb * 128
                            nc.sync.dma_start(xn, x_d[r0:r0 + 128, :])
                            for tq in range(4):
                                pst = psA.tile([128, 512], F32, tag="pst")
                                for j in range(4):
                                    tin = tq * 4 + j
                                    nc.tensor.transpose(
                                        pst[:, j * 128:(j + 1) * 128],
                                        xn[:, tin * 128:(tin + 1) * 128], ident)
                                nc.vector.tensor_copy(
                                    pv3[:, tq * 4:(tq + 1) * 4, sb * 128:sb * 128 + 128],
                                    pst.rearrange("p (a b) -> p a b", a=4))
                        psQ = [psB.tile([128, 512], F32, tag=f"psq{r}", name=f"psq{r}") for r in range(NREP)]
                        psK = psB.tile([128, 512], F32, tag="psk")
                        psV = psB.tile([128, 512], F32, tag="psv")
                        for tin in range(16):
                            rhs = pv3[:, tin, :]
                            for r in range(NREP):
                                nc.tensor.matmul(psQ[r], w3[:, tin, r * 128:(r + 1) * 128],
                                                 rhs, start=(tin == 0), stop=(tin == 15))
                            nc.tensor.matmul(psK, w3[:, tin, 512:640], rhs,
                                             start=(tin == 0), stop=(tin == 15))
                            nc.tensor.matmul(psV, w3[:, tin, 640:768], rhs,
                                             start=(tin == 0), stop=(tin == 15))
                        for r in range(NREP):
                            nc.scalar.copy(xqT3[:, r, sc * 512:(sc + 1) * 512], psQ[r])
                        nc.scalar.copy(xkT[:, sc * 512:(sc + 1) * 512], psK)
                        vts = vfix_pool.tile([128, 512], F32, tag="vts")
                        nc.vector.tensor_copy(vts, psV)
                        for vb in range(4):
                            vps = psA.tile([128, 512], F32, tag="pst")
                            nc.tensor.transpose(vps[:, 0:128],
                                                vts[:, vb * 128:(vb + 1) * 128], ident)
                            tk = sc * 4 + vb
                            nc.vector.tensor_copy(xv0r[:, tk, 0:64], vps[:, 0:64])
                            nc.vector.tensor_copy(xv1r[:, tk, 0:64], vps[:, 64:128])

                # ---------------- Phase B: attention ----------------
                with tc.tile_pool(name="expCT_pool", bufs=1) as ecp, \
                     tc.tile_pool(name="combT_pool", bufs=2) as ctp, \
                     tc.tile_pool(name="eS_pool", bufs=3) as esp, \
                     tc.tile_pool(name="eT_pool", bufs=3) as etp, \
                     tc.tile_pool(name="norm_pool", bufs=4) as nrm, \
                     tc.tile_pool(name="psS", bufs=3, space="PSUM") as psS, \
                     tc.tile_pool(name="psO", bufs=2, space="PSUM") as psO:
                    expCT = ecp.tile([128, TK * S], BF16)
                    expCT3 = expCT.rearrange("p (t q) -> p t q", t=TK)
                    for t in range(TK):
                        combT = ctp.tile([128, S], BF16, tag="combT")
                        nc.sync.dma_start_transpose(combT, comb_d[:, t * 128:(t + 1) * 128])
                        nc.scalar.activation(expCT3[:, t, :], combT,
                                             mybir.ActivationFunctionType.Exp)

                    for g in range(G):
                        tmax = min(g + 1, TK) if causal else TK
                        oP = [psO.tile([128, 512], F32, tag="po", name=f"po{kvl}") for kvl in range(NKVL)]
                        for t0 in range(0, tmax, 2):
                            npair = min(2, tmax - t0)
                            sS = [psS.tile([128, 1024], F32, tag="ps", name=f"ps{kvl}")
                                  for kvl in range(NKVL)]
                            for j in range(npair):
                                t = t0 + j
                                for kvl in range(NKVL):
                                    p0, p1 = kvl * 64, (kvl + 1) * 64
                                    nc.tensor.matmul(
                                        sS[kvl][:, j * 512:(j + 1) * 512],
                                        xkT[p0:p1, t * 128:(t + 1) * 128],
                                        xqT3[p0:p1, :, g * 128:(g + 1) * 128],
                                        start=True, stop=True)
                            for kvl in range(NKVL):
                                w = npair * 512
                                eS = esp.tile([128, 1024], BF16, tag="eS")
                                nc.scalar.activation(eS[:, :w], sS[kvl][:, :w],
                                                     mybir.ActivationFunctionType.Exp,
                                                     scale=0.125)
                                eT = etp.tile([128, 1024], BF16, tag="eT")
                                e_in1 = (expCT3[:, t0:t0 + npair, g * 128:(g + 1) * 128]
                                         .unsqueeze(2).broadcast_to((128, npair, NREP, 128)))
                                nc.vector.tensor_mul(
                                    eT[:, :w].rearrange("p (t h q) -> p t h q", t=npair, h=NREP),
                                    eS[:, :w].rearrange("p (t h q) -> p t h q", t=npair, h=NREP),
                                    e_in1)
                                xvr = xv0r if kvl == 0 else xv1r
                                for j in range(npair):
                                    t = t0 + j
                                    nc.tensor.matmul(
                                        oP[kvl][0:65, :], xvr[:, t, :],
                                        eT[:, j * 512:(j + 1) * 512],
                                        start=(t == 0), stop=(t == tmax - 1))
                        for kvl in range(NKVL):
                            ssum = nrm.tile([1, 512], F32, tag="ssum")
                            nc.vector.tensor_scalar_add(ssum, oP[kvl][64:65, :], 1e-30)
                            rec = nrm.tile([1, 512], F32, tag="rec")
                            nc.vector.reciprocal(rec, ssum)
                            recb = nrm.tile([64, 512], F32, tag="recb")
                            nc.gpsimd.partition_broadcast(recb, rec)
                            rec_b = recb.rearrange("p (h q) -> p h q", h=NREP)
                            src = oP[kvl][0:64, :].rearrange("p (h q) -> p h q", h=NREP)
                            if kvl == 0:
                                nc.vector.tensor_mul(
                                    attnT3[0:64, :, g * 128:(g + 1) * 128], src, rec_b)
                            else:
                                shift = nrm.tile([64, 512], F32, tag="shift")
                                nc.vector.tensor_mul(
                                    shift.rearrange("p (h q) -> p h q", h=NREP), src, rec_b)
                                nc.sync.dma_start(
                                    attnT3[64:128, :, g * 128:(g + 1) * 128],
                                    shift.rearrange("p (h q) -> p h q", h=NREP))

                # ---------------- Phase C: output projection ----------------
                with tc.tile_pool(name="wo_pool", bufs=1) as wop, \
                     tc.tile_pool(name="y_pool", bufs=3) as yp, \
                     tc.tile_pool(name="psY", bufs=2, space="PSUM") as psY:
                    wo_sb = wop.tile([128, NREP * D], F32)
                    wo3 = wo_sb.rearrange("p (r n) -> p r n", r=NREP)
                    nc.sync.dma_start(wo3, wo_d.rearrange("(r p) n -> p r n", p=128))
                    for st in range(S // 128):
                        pY = psY.tile([128, D], F32, tag="py")
                        for r in range(NREP):
                            lhsT = attnT3[:, r, st * 128:(st + 1) * 128]
                            for nch in range(4):
                                nc.tensor.matmul(pY[:, nch * 512:(nch + 1) * 512],
                                                 lhsT, wo3[:, r, nch * 512:(nch + 1) * 512],
                                                 start=(r == 0), stop=(r == NREP - 1))
                        y_sb = yp.tile([128, D], F32, tag="ysb")
                        if st % 2 == 0:
                            nc.vector.tensor_copy(y_sb, pY)
                        else:
                            nc.scalar.copy(y_sb, pY)
                        nc.sync.dma_start(y_d[st * 128:(st + 1) * 128, :], y_sb)

        for _rep in range(loop_n):
            body()

    nc.compile()
    return nc


def shard_inputs(x, mask, graph_bias, wq, wk, wv, wo, S=2048):
    """Build the 8 per-core input maps from the full inputs."""
    mask2 = np.asarray(mask, dtype=np.float32).reshape(S, S)
    gb = np.asarray(graph_bias, dtype=np.float32).reshape(2, S, S)
    comb_b = [(mask2 + gb[b]).astype(ml_dtypes.bfloat16) for b in range(2)]
    x = np.ascontiguousarray(np.asarray(x, dtype=np.float32))
    wq = np.asarray(wq, dtype=np.float32)
    wk = np.asarray(wk, dtype=np.float32)
    wv = np.asarray(wv, dtype=np.float32)
    wo = np.asarray(wo, dtype=np.float32)

    in_maps = []
    for c in range(N_CORES):
        b = c % 2
        kvp = c // 2
        kvg = (2 * kvp, 2 * kvp + 1)
        qcols, orows = [], []
        for r in range(NREP):
            for kv in kvg:
                h = kv * NREP + r
                qcols.extend(range(h * HD, (h + 1) * HD))
                orows.extend(range(h * HD, (h + 1) * HD))
        kcols = []
        for kv in kvg:
            kcols.extend(range(kv * HD, (kv + 1) * HD))
        wqkv = np.concatenate(
            [wq[:, qcols], wk[:, kcols], wv[:, kcols]], axis=1)
        in_maps.append({
            "x": np.ascontiguousarray(x[b]),
            "comb": comb_b[b],
            "wqkv": np.ascontiguousarray(wqkv),
            "wo": np.ascontiguousarray(wo[orows, :]),
        })
    return in_maps


def gather_outputs(results, S=2048):
    y = np.zeros((2, S, D), dtype=np.float32)
    for c in range(N_CORES):
        y[c % 2] += results[c]["y"]
    return y


def detect_causal(mask, graph_bias, S=2048):
    """True if every score tile strictly above the block diagonal is fully
    masked (so the kernel may skip it): those tiles then contribute exactly 0
    probability, matching the reference."""
    if S % 128:
        return False
    m = np.asarray(mask, dtype=np.float32).reshape(S, S)
    nb = S // 128
    blockmax = m.reshape(nb, 128, nb, 128).max(axis=(1, 3))
    upper = np.triu(np.ones((nb, nb), dtype=bool), k=1)
    if not upper.any():
        return False
    if not bool((blockmax[upper] < -1e8).all()):
        return False
    return float(np.abs(np.asarray(graph_bias)).max()) < 1e6


_PROGRAM_CACHE = {}


def _get_program(S, causal, loop_n=1):
    key = (S, causal, loop_n)
    if key not in _PROGRAM_CACHE:
        _PROGRAM_CACHE[key] = build_program(S=S, causal=causal, loop_n=loop_n)
    return _PROGRAM_CACHE[key]


def kernel(x, mask, graph_bias, wq, wk, wv, wo, start_pos=0):
    S = x.shape[1]
    causal = detect_causal(mask, graph_bias, S=S)
    nc = _get_program(S, causal)
    in_maps = shard_inputs(x, mask, graph_bias, wq, wk, wv, wo, S=S)
    res = run_bass_kernel_spmd(nc, in_maps, core_ids=list(range(N_CORES)))
    return gather_outputs(res.results, S=S)
